# revision 1
# baseline (speedup 1.0000x reference)
"""Supervised contrastive loss on 8 Trainium2 NeuronCores.

Reference computation (N=8192, D=128, TAU=0.1, 100 classes):
    xn   = x / ||x||_row
    sim  = xn @ xn.T                      [N, N]
    e    = exp(sim / TAU)
    top  = sum_j e[i,j] * (y_i == y_j)
    down = sum_j e[i,j]
    loss = mean(log(down) - log(top))

Sharding: anchors (rows) split across 8 cores, 1024 rows each. Every core
normalizes + transposes the full embedding matrix (cheap, O(N*D)) so it can
compute its own [1024, 8192] block of the similarity matrix fully on-chip:

  PE  : fp32r GEMM  xiT[:,128i].T @ xnT -> PSUM [128, 2048] chunks
  ACT : e = exp(psum / TAU) -> bf16 SBUF, accum_out = row-sums (down)
  DVE : top = sum(e * (y_j == y_i)) via one fused scalar_tensor_tensor pass

Device outputs per core: per-row top and down sums ([128, 16] f32).
Host does the final (tiny) log / mean reduction.
"""

import sys

import numpy as np

sys.path.insert(0, "/opt/trn_rl_repo")

import ml_dtypes

TAU = 0.1
N, D = 8192, 128
P = 128
NCORES = 8
ROWS = N // NCORES          # 1024 anchor rows per core
IT = ROWS // P              # 8 i-tiles of 128 anchors
MEGA = 8                    # 128-row j-tiles per normalization mega-tile
NMEGA = N // (MEGA * P)     # 8 mega-tiles covering all of x
CH = 2048                   # exp chunk width (4 PSUM banks)
NCH = N // CH               # 4 chunks per i-tile row block
MM_N = 512                  # fp32 matmul moving-operand limit

_PROGRAM = None


def _build_program():
    import concourse.bacc as bacc
    import concourse.bass as bass  # noqa: F401
    import concourse.mybir as mybir
    from concourse import masks
    from concourse.tile import TileContext

    f32 = mybir.dt.float32
    f32r = mybir.dt.float32r
    bf16 = mybir.dt.bfloat16
    AF = mybir.ActivationFunctionType
    AX = mybir.AxisListType
    OP = mybir.AluOpType

    nc = bacc.Bacc("TRN2", target_bir_lowering=False)
    x_h = nc.declare_dram_parameter("x", [N, D], f32, isOutput=False)
    xo_h = nc.declare_dram_parameter("x_own", [ROWS, D], f32, isOutput=False)
    yb_h = nc.declare_dram_parameter("y_bcast", [P, N], bf16, isOutput=False)
    yi_h = nc.declare_dram_parameter("y_own", [P, IT], f32, isOutput=False)
    out_h = nc.declare_dram_parameter("out", [P, 2 * IT * NCH], f32, isOutput=True)

    with TileContext(nc) as tc:
        with tc.tile_pool(name="persist", bufs=1) as persist:
            xnT = persist.tile([P, N], f32r)       # [d, j] normalized, all rows
            xiT = persist.tile([P, ROWS], f32r)    # [d, i] normalized, own rows
            ybc = persist.tile([P, N], bf16)       # y[j] broadcast down partitions
            yis = persist.tile([P, IT], f32)       # y_own as [p, itile]
            outs = persist.tile([P, 2 * IT * NCH], f32)  # [top parts | down parts]
            identity = persist.tile([P, P], f32)

            nc.sync.dma_start(out=ybc[:], in_=yb_h[:, :])
            nc.sync.dma_start(out=yis[:], in_=yi_h[:, :])
            masks.make_identity(nc, identity[:])

            # Norm mega-tiles are emitted interleaved with the main loop
            # (chunk-outer order) so every engine's in-order queue alternates
            # between the two stages: chunk c consumes megas 2c, 2c+1.
            with (
                tc.tile_pool(name="nx", bufs=3) as nxp,
                tc.tile_pool(name="nsc", bufs=3) as nscp,
                tc.tile_pool(name="mpsum", bufs=2, space="PSUM") as mpp,
                tc.tile_pool(name="ep", bufs=3) as ep,
                tc.tile_pool(name="trashp", bufs=1) as trp,
            ):
                def norm_mega(m):
                    if m >= 0:
                        src = x_h[m * MEGA * P : (m + 1) * MEGA * P, :]
                        dst = xnT[:, m * MEGA * P : (m + 1) * MEGA * P]
                    else:
                        src = xo_h[:, :]
                        dst = xiT[:, :]

                    xt = nxp.tile([P, MEGA, D], f32, tag="xt", name=f"xt{m}")
                    nc.gpsimd.dma_start(
                        out=xt[:], in_=src.rearrange("(g p) d -> p g d", p=P)
                    )
                    sq = nxp.tile([P, MEGA, D], f32, tag="sq", name=f"sq{m}")
                    nc.gpsimd.tensor_tensor(out=sq[:], in0=xt[:], in1=xt[:], op=OP.mult)
                    ss = nscp.tile([P, MEGA], f32, tag="ss", name=f"ss{m}")
                    nc.vector.tensor_reduce(out=ss[:], in_=sq[:], axis=AX.X, op=OP.add)
                    lg = nscp.tile([P, MEGA], f32, tag="lg", name=f"lg{m}")
                    nc.scalar.activation(out=lg[:], in_=ss[:], func=AF.Ln)
                    r0 = nscp.tile([P, MEGA], f32, tag="r0", name=f"r0{m}")
                    nc.scalar.activation(out=r0[:], in_=lg[:], func=AF.Exp, scale=-0.5)
                    xn = nxp.tile([P, MEGA, D], f32, tag="xn", name=f"xn{m}")
                    nc.gpsimd.tensor_tensor(
                        out=xn[:],
                        in0=xt[:],
                        in1=r0[:, :].unsqueeze(-1).broadcast_to([P, MEGA, D]),
                        op=OP.mult,
                    )
                    pt = mpp.tile([P, CH], f32, tag="ps", name=f"pt{m}")
                    for g in range(MEGA):
                        nc.tensor.transpose(
                            out=pt[:, g * P : (g + 1) * P],
                            in_=xn[:, g, :],
                            identity=identity[:],
                        )
                    nc.scalar.copy(out=dst, in_=pt[:, : MEGA * P])

                norm_mega(-1)
                norm_mega(0)
                norm_mega(1)
                trash = trp.tile([P, CH], bf16)
                for c in range(NCH):
                    if c < NCH - 1:
                        norm_mega(2 * c + 2)
                        norm_mega(2 * c + 3)
                    for t in range(IT):
                        lhsT = xiT[:, t * P : (t + 1) * P]
                        ps = mpp.tile([P, CH], f32, tag="ps", name=f"ps{t}_{c}")
                        for k in range(CH // MM_N):
                            j0 = c * CH + k * MM_N
                            nc.tensor.matmul(
                                out=ps[:, k * MM_N : (k + 1) * MM_N],
                                lhsT=lhsT,
                                rhs=xnT[:, j0 : j0 + MM_N],
                                start=True,
                                stop=True,
                            )
                        e = ep.tile([P, CH], f32, tag="e", name=f"e{t}_{c}")
                        nc.scalar.activation(
                            out=e[:],
                            in_=ps[:],
                            func=AF.Exp,
                            scale=1.0 / TAU,
                            accum_out=outs[:, IT * NCH + t * NCH + c :
                                            IT * NCH + t * NCH + c + 1],
                        )
                        nc.vector.scalar_tensor_tensor(
                            out=trash[:],
                            in0=ybc[:, c * CH : (c + 1) * CH],
                            scalar=yis[:, t : t + 1],
                            in1=e[:],
                            op0=OP.is_equal,
                            op1=OP.mult,
                            accum_out=outs[:, t * NCH + c : t * NCH + c + 1],
                        )
            nc.gpsimd.dma_start(out=out_h[:, :], in_=outs[:])
    nc.compile()
    return nc


def _get_program():
    global _PROGRAM
    if _PROGRAM is None:
        _PROGRAM = _build_program()
    return _PROGRAM


def make_in_maps(x, y):
    x = np.ascontiguousarray(x, dtype=np.float32)
    yf = np.asarray(y).astype(np.float32)
    ybc = np.ascontiguousarray(
        np.broadcast_to(yf.astype(ml_dtypes.bfloat16)[None, :], (P, N))
    )
    in_maps = []
    for c in range(NCORES):
        rows = slice(c * ROWS, (c + 1) * ROWS)
        # y_own[p, t] = y[c*1024 + t*128 + p]
        yi = np.ascontiguousarray(yf[rows].reshape(IT, P).T)
        in_maps.append(
            {
                "x": x,
                "x_own": np.ascontiguousarray(x[rows]),
                "y_bcast": ybc,
                "y_own": yi,
            }
        )
    return in_maps


def finalize(per_core_outs):
    """per_core_outs: list of 8 [P, 2*IT*NCH] f32 arrays -> scalar loss."""
    tops = np.empty((NCORES, IT, P), np.float64)
    downs = np.empty((NCORES, IT, P), np.float64)
    for c, o in enumerate(per_core_outs):
        o = np.asarray(o, dtype=np.float64)
        t = o[:, : IT * NCH].reshape(P, IT, NCH).sum(axis=2)    # [P, IT]
        d = o[:, IT * NCH :].reshape(P, IT, NCH).sum(axis=2)
        tops[c] = t.T
        downs[c] = d.T
    top = tops.reshape(-1)
    down = downs.reshape(-1)
    return np.float32(np.mean(np.log(down) - np.log(top)))


def kernel(x, y):
    from concourse.bass_utils import run_bass_kernel_spmd

    nc = _get_program()
    in_maps = make_in_maps(x, y)
    res = run_bass_kernel_spmd(nc, in_maps, list(range(NCORES)))
    return finalize([r["out"] for r in res.results])



# revision 14
# speedup vs baseline: 1.7256x; 1.7256x over previous
"""Supervised contrastive loss on 8 Trainium2 NeuronCores.

Reference (N=8192, D=128, TAU=0.1, 100 classes):
    xn   = x / ||x||_row
    e    = exp(xn @ xn.T / TAU)
    top  = sum_j e[i,j] * (y_i == y_j)
    down = sum_j e[i,j]
    loss = mean(log(down) - log(top))

Strategy (cyclic-symmetric decomposition, one uniform program for all cores):
  * Host sorts rows by class (loss is permutation invariant) so all
    same-class pairs live within a 3-tile band of the diagonal.
  * e is symmetric, so only half the matrix is computed: each 128-row tile t
    computes a cyclic window of the 33 (t<32) or 32 (t>=32) j-tiles starting
    at its own diagonal. Row sums give `down` for the tile's rows; mirror
    column sums of each computed 128x128 cell (via a stationary-weights
    matmul against a ones vector, accumulated in a tiny [128,64] PSUM tile)
    give `down` for the transposed pairs. top = masked row sums + masked
    mirror column sums over the first 3 window tiles, with the mask shipped
    from the host as a bf16 tensor.
  * Core c owns global tiles {8k+c}. Its x input is pre-rotated by 128*c
    rows so every core runs the identical instruction stream.
  * All heavy data is bf16 (PE matmul runs 1 col/cycle, DVE gets 2-4x modes).
    rsqrt for normalization runs on DVE via a quadratic seed + 2 Newton
    steps, so ACT only ever uses the Exp table (one table load).
  * Host reassembles per-row/per-column partial sums in f64 and takes the
    final log/mean.
"""

import sys

import numpy as np

sys.path.insert(0, "/opt/trn_rl_repo")

import ml_dtypes

TAU = 0.1
N, D = 8192, 128
P = 128
NCORES = 8
NT = N // P              # 64 global row tiles
KT = NT // NCORES        # 8 own tiles per core
BANDT = 3                # masked band tiles per window
BW = BANDT * P           # 384 mask cols per tile
MEGA = 8                 # row-tiles per normalization mega tile
NMEGA = NT // MEGA       # 8 megas
CH = 1536                # psum/exp chunk width (3 banks)
MM_N = 512               # max moving cols per matmul

# rsqrt seed: minimax-ish quadratic fit of z^-0.5 over the ||x||^2 range
_zg = np.linspace(40.0, 280.0, 2001)
_C2, _C1, _C0 = np.polyfit(_zg, 1.0 / np.sqrt(_zg), 2)

_PROGRAM = None


def _win_tiles(tg):
    """window width in tiles for global tile tg (incl. diagonal tile)."""
    return 33 if tg < NT // 2 else 32


def _chunks_for_tile(k):
    """[(chunk_cols, [(psum_off, xnT_col, width), ...]), ...] for own tile k.
    Window starts at local col 1024k, wraps mod N. All 128-aligned."""
    tg_width = _win_tiles(8 * k) * P  # uniform across cores: 8k+c < 32 iff k < 4
    start = 1024 * k
    chunks = []
    done = 0
    while done < tg_width:
        cw = min(CH, tg_width - done)
        mms = []
        off = 0
        while off < cw:
            col = (start + done + off) % N
            w = min(MM_N, cw - off, N - col)
            mms.append((off, col, w))
            off += w
        chunks.append((cw, mms))
        done += cw
    return chunks


def _n_downc_slots():
    return sum(_win_tiles(8 * k) - 1 for k in range(KT))


def _build_program():
    import concourse.bacc as bacc
    import concourse.mybir as mybir
    from concourse import masks
    from concourse.tile import TileContext

    f32 = mybir.dt.float32
    bf16 = mybir.dt.bfloat16
    AF = mybir.ActivationFunctionType
    AX = mybir.AxisListType
    OP = mybir.AluOpType

    nc = bacc.Bacc("TRN2", target_bir_lowering=False)
    x_h = nc.declare_dram_parameter("x", [N, D], bf16, isOutput=False)
    m_h = nc.declare_dram_parameter("m", [P, KT * BW], bf16, isOutput=False)
    topr_h = nc.declare_dram_parameter("topr", [P, KT], f32, isOutput=True)
    downr_h = nc.declare_dram_parameter("downr", [P, 3 * KT], f32, isOutput=True)
    downc_h = nc.declare_dram_parameter(
        "downc", [P, _n_downc_slots()], f32, isOutput=True
    )
    topc_h = nc.declare_dram_parameter(
        "topc", [P, (BANDT - 1) * KT], f32, isOutput=True
    )

    # each (k, w) mirror colsum gets its own psum slot; host sums per column
    slot_of = {}
    for k in range(KT):
        for w in range(1, _win_tiles(8 * k)):
            slot_of[(k, w)] = len(slot_of)
    n_slots = len(slot_of)
    assert n_slots == _n_downc_slots()

    with TileContext(nc) as tc:
        with (
            tc.tile_pool(name="persist", bufs=1) as pp,
            tc.tile_pool(name="acc", bufs=1, space="PSUM") as accp,
        ):
            xnT = pp.tile([P, N], bf16)
            xt = pp.tile([P, NMEGA, MEGA, D], bf16)     # raw rows, all megas
            mt = pp.tile([P, KT * BW], bf16)
            ss = pp.tile([P, NT], bf16)                 # ||x||^2 per row tile
            r0 = pp.tile([P, NT], f32)                  # 1/||x||
            topr = pp.tile([P, KT], f32)
            downr = pp.tile([P, 3 * KT], f32)
            acc_sb = pp.tile([P, n_slots + (BANDT - 1) * KT], f32)
            ones = pp.tile([P, 1], bf16)
            identity = pp.tile([P, P], bf16)
            acc = accp.tile([P, n_slots + (BANDT - 1) * KT], f32)

            nc.vector.memset(ones[:], 1.0)
            masks.make_identity(nc, identity[:])
            nc.sync.dma_start(out=mt[:], in_=m_h[:, :])
            for m in range(NMEGA):
                nc.sync.dma_start(
                    out=xt[:, m],
                    in_=x_h[m * MEGA * P : (m + 1) * MEGA * P, :].rearrange(
                        "(g p) d -> p g d", p=P
                    ),
                )

            with (
                tc.tile_pool(name="sq", bufs=2) as sqp,
                tc.tile_pool(name="nr", bufs=2) as nrp,
                tc.tile_pool(name="xn", bufs=3) as xnp,
                tc.tile_pool(name="tp", bufs=1, space="PSUM") as tpp,
                tc.tile_pool(name="mm", bufs=2, space="PSUM") as mmp,
                tc.tile_pool(name="ep", bufs=3) as ep,
                tc.tile_pool(name="emp", bufs=2) as emp,
            ):
                def norm_scale(m):
                    """scale mega m by r0, PE-transpose, copy into xnT."""
                    eng = nc.vector if m < 5 else nc.gpsimd
                    xn = xnp.tile([P, MEGA, D], bf16, tag="xn", name=f"xn{m}")
                    eng.tensor_tensor(
                        out=xn[:],
                        in0=xt[:, m],
                        in1=r0[:, m * MEGA : (m + 1) * MEGA]
                        .unsqueeze(-1)
                        .broadcast_to([P, MEGA, D]),
                        op=OP.mult,
                    )
                    pt = tpp.tile([P, MEGA * P], bf16, tag="pt", name=f"pt{m}")
                    for g in range(MEGA):
                        nc.tensor.transpose(
                            out=pt[:, g * P : (g + 1) * P],
                            in_=xn[:, g, :],
                            identity=identity[:],
                        )
                    nc.vector.tensor_copy(
                        out=xnT[:, m * MEGA * P : (m + 1) * MEGA * P], in_=pt[:]
                    )

                def newton_rsqrt(mlo):
                    """r0[:, 8mlo:8mlo+16] = rsqrt(ss[...]) for megas mlo, mlo+1."""
                    s = slice(mlo * MEGA, (mlo + 2) * MEGA)
                    z = ss[:, s]
                    y = r0[:, s]
                    t1 = nrp.tile([P, 2 * MEGA], f32, tag="t1", name=f"t1{mlo}")
                    t2 = nrp.tile([P, 2 * MEGA], f32, tag="t2", name=f"t2{mlo}")
                    nc.vector.tensor_scalar(
                        out=t1[:], in0=z, scalar1=float(_C1), scalar2=float(_C0),
                        op0=OP.mult, op1=OP.add,
                    )
                    nc.vector.tensor_tensor(out=t2[:], in0=z, in1=z, op=OP.mult)
                    nc.vector.scalar_tensor_tensor(
                        out=y, in0=t2[:], scalar=float(_C2), in1=t1[:],
                        op0=OP.mult, op1=OP.add,
                    )
                    for _ in range(2):
                        nc.vector.tensor_tensor(out=t1[:], in0=y, in1=y, op=OP.mult)
                        nc.vector.tensor_tensor(out=t2[:], in0=t1[:], in1=z, op=OP.mult)
                        nc.vector.tensor_scalar(
                            out=t1[:], in0=t2[:], scalar1=-0.5, scalar2=1.5,
                            op0=OP.mult, op1=OP.add,
                        )
                        nc.vector.tensor_tensor(out=y, in0=y, in1=t1[:], op=OP.mult)

                for m in range(NMEGA):
                    sq = sqp.tile([P, MEGA, D], bf16, tag="sq", name=f"sq{m}")
                    nc.vector.tensor_tensor(
                        out=sq[:], in0=xt[:, m], in1=xt[:, m], op=OP.mult
                    )
                    with nc.allow_low_precision("bf16 row-norm sums, 0.4% rel"):
                        nc.vector.tensor_reduce(
                            out=ss[:, m * MEGA : (m + 1) * MEGA],
                            in_=sq[:],
                            axis=AX.X,
                            op=OP.add,
                        )
                    if m % 2 == 1:
                        newton_rsqrt(m - 1)
                        norm_scale(m - 1)
                        norm_scale(m)

                # main loop: per own tile, gemm window chunks -> exp -> sums
                pending = []  # delayed colsum emission for PE overlap

                def emit_pending():
                    for fn in pending:
                        fn()
                    pending.clear()

                for k in range(KT):
                    lhsT = xnT[:, 1024 * k : 1024 * k + P]
                    for j, (cw, mms) in enumerate(_chunks_for_tile(k)):
                        ps = mmp.tile([P, CH], f32, tag="ps", name=f"ps{k}_{j}")
                        for off, col, w in mms:
                            nc.tensor.matmul(
                                out=ps[:, off : off + w],
                                lhsT=lhsT,
                                rhs=xnT[:, col : col + w],
                                start=True,
                                stop=True,
                            )
                        emit_pending()
                        e = ep.tile([P, CH], bf16, tag="e", name=f"e{k}_{j}")
                        nc.scalar.activation(
                            out=e[:, :cw],
                            in_=ps[:, :cw],
                            func=AF.Exp,
                            scale=1.0 / TAU,
                            accum_out=downr[:, 3 * k + j : 3 * k + j + 1],
                        )
                        if j == 0:
                            em = emp.tile([P, BW], bf16, tag="em", name=f"em{k}")
                            nc.vector.scalar_tensor_tensor(
                                out=em[:],
                                in0=mt[:, k * BW : (k + 1) * BW],
                                scalar=1.0,
                                in1=e[:, :BW],
                                op0=OP.mult,
                                op1=OP.mult,
                                accum_out=topr[:, k : k + 1],
                            )

                            def top_cols(k=k, em=em):
                                for w in range(1, BANDT):
                                    s = n_slots + (BANDT - 1) * k + w - 1
                                    nc.tensor.matmul(
                                        out=acc[:, s : s + 1],
                                        lhsT=em[:, w * P : (w + 1) * P],
                                        rhs=ones[:],
                                        start=True,
                                        stop=True,
                                    )

                            pending.append(top_cols)

                        def down_cols(k=k, j=j, cw=cw, e=e):
                            for wo in range(0, cw, P):
                                w = (j * CH + wo) // P
                                if w == 0:
                                    continue  # diagonal tile: rows cover it
                                s = slot_of[(k, w)]
                                nc.tensor.matmul(
                                    out=acc[:, s : s + 1],
                                    lhsT=e[:, wo : wo + P],
                                    rhs=ones[:],
                                    start=True,
                                    stop=True,
                                )

                        pending.append(down_cols)
                emit_pending()

            nc.vector.tensor_copy(out=acc_sb[:], in_=acc[:])
            nc.sync.dma_start(out=topr_h[:, :], in_=topr[:])
            nc.sync.dma_start(out=downr_h[:, :], in_=downr[:])
            nc.sync.dma_start(out=downc_h[:, :], in_=acc_sb[:, :n_slots])
            nc.sync.dma_start(
                out=topc_h[:, :], in_=acc_sb[:, n_slots : n_slots + (BANDT - 1) * KT]
            )
    nc.compile()
    return nc


def _get_program():
    global _PROGRAM
    if _PROGRAM is None:
        _PROGRAM = _build_program()
    return _PROGRAM


def make_in_maps(x, y):
    x = np.asarray(x, dtype=np.float32)
    y = np.asarray(y)
    perm = np.argsort(y, kind="stable")
    xs = np.ascontiguousarray(x[perm]).astype(ml_dtypes.bfloat16)
    ys = np.asarray(y)[perm].astype(np.int64)

    # class spans must fit the BANDT-tile mask band
    starts = np.searchsorted(ys, np.unique(ys), side="left")
    ends = np.searchsorted(ys, np.unique(ys), side="right")
    assert np.max((ends - 1) // P - starts // P) <= BANDT - 1, (
        "class span exceeds mask band; raise BANDT"
    )

    in_maps = []
    for c in range(NCORES):
        rot = P * c
        xr = np.ascontiguousarray(np.roll(xs, -rot, axis=0))
        yl = np.roll(ys, -rot)
        m = np.zeros((P, KT * BW), dtype=ml_dtypes.bfloat16)
        for k in range(KT):
            rcls = yl[1024 * k : 1024 * k + P]          # own tile k rows
            ccls = yl[(1024 * k + np.arange(BW)) % N]   # band cols
            m[:, k * BW : (k + 1) * BW] = (
                rcls[:, None] == ccls[None, :]
            ).astype(ml_dtypes.bfloat16)
        in_maps.append({"x": xr, "m": m})
    return in_maps


def finalize(results):
    """results: list of 8 dicts with topr/downr/downc/topc -> scalar loss."""
    slot_kw = []
    for k in range(KT):
        for w in range(1, _win_tiles(8 * k)):
            slot_kw.append((k, w))

    down = np.zeros(N, np.float64)
    top = np.zeros(N, np.float64)
    for c, r in enumerate(results):
        topr = np.asarray(r["topr"], np.float64)
        downr = np.asarray(r["downr"], np.float64)
        downc = np.asarray(r["downc"], np.float64)
        topc = np.asarray(r["topc"], np.float64)
        p = np.arange(P)
        for k in range(KT):
            gl = P * (8 * k + c) + p
            down[gl] += downr[:, 3 * k : 3 * k + 3].sum(axis=1)
            top[gl] += topr[:, k]
            for w in range(1, BANDT):
                vloc = 8 * k + w
                gl2 = P * ((vloc + c) % NT) + p
                top[gl2] += topc[:, (BANDT - 1) * k + w - 1]
        for s, (k, w) in enumerate(slot_kw):
            vloc = (8 * k + w) % NT
            gl = P * ((vloc + c) % NT) + p
            down[gl] += downc[:, s]
    return np.float32(np.mean(np.log(down) - np.log(top)))


def kernel(x, y):
    from concourse.bass_utils import run_bass_kernel_spmd

    nc = _get_program()
    in_maps = make_in_maps(x, y)
    res = run_bass_kernel_spmd(nc, in_maps, list(range(NCORES)))
    return finalize(res.results)


# revision 15
# speedup vs baseline: 1.9633x; 1.1378x over previous
"""Supervised contrastive loss on 8 Trainium2 NeuronCores.

Reference (N=8192, D=128, TAU=0.1, 100 classes):
    xn   = x / ||x||_row
    e    = exp(xn @ xn.T / TAU)
    top  = sum_j e[i,j] * (y_i == y_j)
    down = sum_j e[i,j]
    loss = mean(log(down) - log(top))

Strategy (cyclic-symmetric decomposition, one uniform program for all cores):
  * Host sorts rows by class (the loss is permutation invariant) so all
    same-class pairs live within a 3-tile band of the diagonal, normalizes
    rows in f32, and ships xn as bf16.
  * e is symmetric, so only half the matrix is computed: each 128-row tile
    computes a cyclic window of 33 (tiles 0-31) or 32 (tiles 32-63) j-tiles
    starting at its own diagonal. Row sums (ACT exp accumulator) give `down`
    for the tile's rows; mirror column sums of every off-diagonal 128x128
    cell - a stationary-weights matmul of the bf16 e-cell against a ones
    vector into a private [128,1] PSUM slot - give `down` for the mirrored
    pairs. top = masked row sums (DVE scalar_tensor_tensor with a host-built
    bf16 class-equality mask) plus masked mirror column sums over window
    tiles 1..2.
  * Core c owns global tiles {8k+c}; its input is pre-rotated by 128*c rows
    so one instruction stream serves all 8 cores (SPMD).
  * Everything heavy is bf16; ACT uses only the Exp table (one table load).
  * The host reassembles the per-row / per-column partial sums in f64.
"""

import sys

import numpy as np

sys.path.insert(0, "/opt/trn_rl_repo")

import ml_dtypes

TAU = 0.1
N, D = 8192, 128
P = 128
NCORES = 8
NT = N // P              # 64 global row tiles
KT = NT // NCORES        # 8 own tiles per core
BANDT = 3                # masked band tiles per window
BW = BANDT * P           # 384 mask cols per tile
MEGA = 8                 # row-tiles per transpose mega tile
NMEGA = NT // MEGA       # 8 megas
CH = 1536                # psum/exp chunk width (3 banks)
MM_N = 512               # max moving cols per matmul

_PROGRAM = None


def _win_tiles(tg):
    """window width in tiles for global tile tg (incl. diagonal tile)."""
    return 33 if tg < NT // 2 else 32


def _chunks_for_tile(k):
    """[(chunk_cols, [(psum_off, xnT_col, width), ...]), ...] for own tile k.
    Window starts at local col 1024k, wraps mod N. All 128-aligned."""
    tg_width = _win_tiles(8 * k) * P  # uniform across cores: 8k+c < 32 iff k < 4
    start = 1024 * k
    chunks = []
    done = 0
    while done < tg_width:
        cw = min(CH, tg_width - done)
        mms = []
        off = 0
        while off < cw:
            col = (start + done + off) % N
            w = min(MM_N, cw - off, N - col)
            mms.append((off, col, w))
            off += w
        chunks.append((cw, mms))
        done += cw
    return chunks


def _n_downc_slots():
    return sum(_win_tiles(8 * k) - 1 for k in range(KT))


def _build_program():
    import concourse.bacc as bacc
    import concourse.mybir as mybir
    from concourse import masks
    from concourse.tile import TileContext

    f32 = mybir.dt.float32
    bf16 = mybir.dt.bfloat16
    AF = mybir.ActivationFunctionType
    OP = mybir.AluOpType

    nc = bacc.Bacc("TRN2", target_bir_lowering=False)
    x_h = nc.declare_dram_parameter("x", [N, D], bf16, isOutput=False)
    m_h = nc.declare_dram_parameter("m", [P, KT * BW], bf16, isOutput=False)
    topr_h = nc.declare_dram_parameter("topr", [P, KT], f32, isOutput=True)
    downr_h = nc.declare_dram_parameter("downr", [P, 3 * KT], f32, isOutput=True)
    downc_h = nc.declare_dram_parameter(
        "downc", [P, _n_downc_slots()], f32, isOutput=True
    )
    topc_h = nc.declare_dram_parameter(
        "topc", [P, (BANDT - 1) * KT], f32, isOutput=True
    )

    # each (k, w) mirror colsum gets its own psum slot; host sums per column
    slot_of = {}
    for k in range(KT):
        for w in range(1, _win_tiles(8 * k)):
            slot_of[(k, w)] = len(slot_of)
    n_slots = len(slot_of)

    # chunk emission schedule: a chunk is ready once the megas its columns
    # (and its lhsT tile) live in have been transposed into xnT
    sched = {m: [] for m in range(NMEGA)}
    for k in range(KT):
        for j, (cw, mms) in enumerate(_chunks_for_tile(k)):
            need = {k}
            for off, col, w in mms:
                need.add(col // (MEGA * P))
                need.add((col + w - 1) // (MEGA * P))
            sched[max(need)].append((k, j, cw, mms))

    with TileContext(nc) as tc:
        with (
            tc.tile_pool(name="persist", bufs=1) as pp,
            tc.tile_pool(name="acc", bufs=1, space="PSUM") as accp,
        ):
            xnT = pp.tile([P, N], bf16)
            mt = pp.tile([P, KT * BW], bf16)
            topr = pp.tile([P, KT], f32)
            downr = pp.tile([P, 3 * KT], f32)
            acc_sb = pp.tile([P, n_slots + (BANDT - 1) * KT], f32)
            ones = pp.tile([P, 1], bf16)
            identity = pp.tile([P, P], bf16)
            acc = accp.tile([P, n_slots + (BANDT - 1) * KT], f32)

            nc.vector.memset(ones[:], 1.0)
            masks.make_identity(nc, identity[:])
            nc.sync.dma_start(out=mt[:], in_=m_h[:, :])

            with (
                tc.tile_pool(name="xt", bufs=3) as xtp,
                tc.tile_pool(name="tp", bufs=1, space="PSUM") as tpp,
                tc.tile_pool(name="mm", bufs=2, space="PSUM") as mmp,
                tc.tile_pool(name="ep", bufs=4) as ep,
                tc.tile_pool(name="emp", bufs=2) as emp,
            ):
                pending = []  # delayed colsum emission for PE overlap

                def emit_pending():
                    for fn in pending:
                        fn()
                    pending.clear()

                def emit_mega(m):
                    xt = xtp.tile([P, MEGA, D], bf16, tag="xt", name=f"xt{m}")
                    nc.sync.dma_start(
                        out=xt[:],
                        in_=x_h[m * MEGA * P : (m + 1) * MEGA * P, :].rearrange(
                            "(g p) d -> p g d", p=P
                        ),
                    )
                    pt = tpp.tile([P, MEGA * P], bf16, tag="pt", name=f"pt{m}")
                    for g in range(MEGA):
                        nc.tensor.transpose(
                            out=pt[:, g * P : (g + 1) * P],
                            in_=xt[:, g, :],
                            identity=identity[:],
                        )
                    nc.vector.tensor_copy(
                        out=xnT[:, m * MEGA * P : (m + 1) * MEGA * P], in_=pt[:]
                    )

                def emit_chunk(k, j, cw, mms):
                    lhsT = xnT[:, 1024 * k : 1024 * k + P]
                    ps = mmp.tile([P, CH], f32, tag="ps", name=f"ps{k}_{j}")
                    for off, col, w in mms:
                        nc.tensor.matmul(
                            out=ps[:, off : off + w],
                            lhsT=lhsT,
                            rhs=xnT[:, col : col + w],
                            start=True,
                            stop=True,
                        )
                    emit_pending()
                    e = ep.tile([P, CH], bf16, tag="e", name=f"e{k}_{j}")
                    nc.scalar.activation(
                        out=e[:, :cw],
                        in_=ps[:, :cw],
                        func=AF.Exp,
                        scale=1.0 / TAU,
                        accum_out=downr[:, 3 * k + j : 3 * k + j + 1],
                    )
                    if j == 0:
                        em = emp.tile([P, BW], bf16, tag="em", name=f"em{k}")
                        nc.vector.scalar_tensor_tensor(
                            out=em[:],
                            in0=mt[:, k * BW : (k + 1) * BW],
                            scalar=1.0,
                            in1=e[:, :BW],
                            op0=OP.mult,
                            op1=OP.mult,
                            accum_out=topr[:, k : k + 1],
                        )

                        def top_cols(k=k, em=em):
                            for w in range(1, BANDT):
                                s = n_slots + (BANDT - 1) * k + w - 1
                                nc.tensor.matmul(
                                    out=acc[:, s : s + 1],
                                    lhsT=em[:, w * P : (w + 1) * P],
                                    rhs=ones[:],
                                    start=True,
                                    stop=True,
                                )

                        pending.append(top_cols)

                    def down_cols(k=k, j=j, cw=cw, e=e):
                        for wo in range(0, cw, P):
                            w = (j * CH + wo) // P
                            if w == 0:
                                continue  # diagonal tile: rows cover it
                            s = slot_of[(k, w)]
                            nc.tensor.matmul(
                                out=acc[:, s : s + 1],
                                lhsT=e[:, wo : wo + P],
                                rhs=ones[:],
                                start=True,
                                stop=True,
                            )

                    pending.append(down_cols)

                for m in range(NMEGA):
                    emit_mega(m)
                    for k, j, cw, mms in sorted(sched[m]):
                        emit_chunk(k, j, cw, mms)
                emit_pending()

            nc.vector.tensor_copy(out=acc_sb[:], in_=acc[:])
            nc.sync.dma_start(out=topr_h[:, :], in_=topr[:])
            nc.sync.dma_start(out=downr_h[:, :], in_=downr[:])
            nc.sync.dma_start(out=downc_h[:, :], in_=acc_sb[:, :n_slots])
            nc.sync.dma_start(
                out=topc_h[:, :], in_=acc_sb[:, n_slots : n_slots + (BANDT - 1) * KT]
            )
    nc.compile()
    return nc


def _get_program():
    global _PROGRAM
    if _PROGRAM is None:
        _PROGRAM = _build_program()
    return _PROGRAM


def make_in_maps(x, y):
    x = np.asarray(x, dtype=np.float32)
    y = np.asarray(y)
    perm = np.argsort(y, kind="stable")
    xs = np.ascontiguousarray(x[perm])
    xs = xs / np.linalg.norm(xs, axis=-1, keepdims=True)
    xs = xs.astype(ml_dtypes.bfloat16)
    ys = np.asarray(y)[perm].astype(np.int64)

    # class spans must fit the BANDT-tile mask band
    uniq = np.unique(ys)
    starts = np.searchsorted(ys, uniq, side="left")
    ends = np.searchsorted(ys, uniq, side="right")
    assert np.max((ends - 1) // P - starts // P) <= BANDT - 1, (
        "class span exceeds mask band; raise BANDT"
    )

    in_maps = []
    for c in range(NCORES):
        rot = P * c
        xr = np.ascontiguousarray(np.roll(xs, -rot, axis=0))
        yl = np.roll(ys, -rot)
        m = np.zeros((P, KT * BW), dtype=ml_dtypes.bfloat16)
        for k in range(KT):
            rcls = yl[1024 * k : 1024 * k + P]          # own tile k rows
            ccls = yl[(1024 * k + np.arange(BW)) % N]   # band cols
            m[:, k * BW : (k + 1) * BW] = (
                rcls[:, None] == ccls[None, :]
            ).astype(ml_dtypes.bfloat16)
        in_maps.append({"x": xr, "m": m})
    return in_maps


def finalize(results):
    """results: list of 8 dicts with topr/downr/downc/topc -> scalar loss."""
    slot_kw = []
    for k in range(KT):
        for w in range(1, _win_tiles(8 * k)):
            slot_kw.append((k, w))

    down = np.zeros(N, np.float64)
    top = np.zeros(N, np.float64)
    for c, r in enumerate(results):
        topr = np.asarray(r["topr"], np.float64)
        downr = np.asarray(r["downr"], np.float64)
        downc = np.asarray(r["downc"], np.float64)
        topc = np.asarray(r["topc"], np.float64)
        p = np.arange(P)
        for k in range(KT):
            gl = P * (8 * k + c) + p
            down[gl] += downr[:, 3 * k : 3 * k + 3].sum(axis=1)
            top[gl] += topr[:, k]
            for w in range(1, BANDT):
                vloc = 8 * k + w
                gl2 = P * ((vloc + c) % NT) + p
                top[gl2] += topc[:, (BANDT - 1) * k + w - 1]
        for s, (k, w) in enumerate(slot_kw):
            vloc = (8 * k + w) % NT
            gl = P * ((vloc + c) % NT) + p
            down[gl] += downc[:, s]
    return np.float32(np.mean(np.log(down) - np.log(top)))


def kernel(x, y):
    from concourse.bass_utils import run_bass_kernel_spmd

    nc = _get_program()
    in_maps = make_in_maps(x, y)
    res = run_bass_kernel_spmd(nc, in_maps, list(range(NCORES)))
    return finalize(res.results)


# revision 22
# speedup vs baseline: 2.1623x; 1.1013x over previous
"""Supervised contrastive loss on 8 Trainium2 NeuronCores.

Reference (N=8192, D=128, TAU=0.1, 100 classes):
    xn   = x / ||x||_row
    e    = exp(xn @ xn.T / TAU)
    top  = sum_j e[i,j] * (y_i == y_j)
    down = sum_j e[i,j]
    loss = mean(log(down) - log(top))

Strategy (cyclic-symmetric decomposition, one uniform program for all cores):
  * Host sorts rows by class (the loss is permutation invariant) so all
    same-class pairs live within a 3-tile band of the diagonal, normalizes
    rows in f32, and ships xn as bf16.
  * e is symmetric, so only half the matrix is computed: each 128-row tile
    computes a cyclic window of 33 (tiles 0-31) or 32 (tiles 32-63) j-tiles
    starting at its own diagonal. Row sums (ACT exp accumulator) give `down`
    for the tile's rows; mirror column sums of every off-diagonal 128x128
    cell - a stationary-weights matmul of the bf16 e-cell against a ones
    vector into a private [128,1] PSUM slot - give `down` for the mirrored
    pairs. top = masked row sums (DVE scalar_tensor_tensor with a host-built
    bf16 class-equality mask) plus masked mirror column sums over window
    tiles 1..2.
  * Core c owns global tiles {8k+c}; its input is pre-rotated by 128*c rows
    so one instruction stream serves all 8 cores (SPMD).
  * Everything heavy is bf16; ACT uses only the Exp table (one table load).
  * The host reassembles the per-row / per-column partial sums in f64.
"""

import sys

import numpy as np

sys.path.insert(0, "/opt/trn_rl_repo")

import ml_dtypes

TAU = 0.1
N, D = 8192, 128
P = 128
NCORES = 8
NT = N // P              # 64 global row tiles
KT = NT // NCORES        # 8 own tiles per core
BANDT = 3                # masked band tiles per window
BW = BANDT * P           # 384 mask cols per tile
MEGA = 8                 # row-tiles per transpose mega tile
NMEGA = NT // MEGA       # 8 megas
CH = 1536                # psum/exp chunk width (3 banks)
MM_N = 512               # max moving cols per matmul

_PROGRAM = None


def _win_tiles(tg):
    """window width in tiles for global tile tg (incl. diagonal tile)."""
    return 33 if tg < NT // 2 else 32


def _chunks_for_tile(k):
    """[(chunk_cols, [(psum_off, xnT_col, width), ...]), ...] for own tile k.
    Window starts at local col 1024k, wraps mod N. All 128-aligned."""
    tg_width = _win_tiles(8 * k) * P  # uniform across cores: 8k+c < 32 iff k < 4
    start = 1024 * k
    chunks = []
    done = 0
    while done < tg_width:
        cw = min(CH, tg_width - done)
        mms = []
        off = 0
        while off < cw:
            col = (start + done + off) % N
            w = min(MM_N, cw - off, N - col)
            mms.append((off, col, w))
            off += w
        chunks.append((cw, mms))
        done += cw
    return chunks


def _n_downc_slots():
    return sum(_win_tiles(8 * k) - 1 for k in range(KT))


def _build_program():
    import concourse.bacc as bacc
    import concourse.mybir as mybir
    from concourse import masks
    from concourse.tile import TileContext

    f32 = mybir.dt.float32
    bf16 = mybir.dt.bfloat16
    AF = mybir.ActivationFunctionType
    AX = mybir.AxisListType
    OP = mybir.AluOpType

    nc = bacc.Bacc("TRN2", target_bir_lowering=False)
    x_h = nc.declare_dram_parameter("x", [N, D], bf16, isOutput=False)
    m_h = nc.declare_dram_parameter("m", [P, KT * BW], bf16, isOutput=False)
    topr_h = nc.declare_dram_parameter("topr", [P, KT], f32, isOutput=True)
    downr_h = nc.declare_dram_parameter("downr", [P, 3 * KT], f32, isOutput=True)
    downc_h = nc.declare_dram_parameter(
        "downc", [P, _n_downc_slots()], f32, isOutput=True
    )
    topc_h = nc.declare_dram_parameter(
        "topc", [P, (BANDT - 1) * KT], f32, isOutput=True
    )

    # each (k, w) mirror colsum gets its own psum slot; host sums per column
    slot_of = {}
    for k in range(KT):
        for w in range(1, _win_tiles(8 * k)):
            slot_of[(k, w)] = len(slot_of)
    n_slots = len(slot_of)

    # chunk emission schedule: a chunk is ready once the megas its columns
    # (and its lhsT tile) live in have been transposed into xnT
    sched = {m: [] for m in range(NMEGA)}
    for k in range(KT):
        for j, (cw, mms) in enumerate(_chunks_for_tile(k)):
            need = {k}
            for off, col, w in mms:
                need.add(col // (MEGA * P))
                need.add((col + w - 1) // (MEGA * P))
            sched[max(need)].append((k, j, cw, mms))

    with TileContext(nc) as tc:
        with (
            tc.tile_pool(name="persist", bufs=1) as pp,
            tc.tile_pool(name="acc", bufs=1, space="PSUM") as accp,
        ):
            xnT = pp.tile([P, N], bf16)
            mt = pp.tile([P, KT * BW], bf16)
            topr = pp.tile([P, KT], f32)
            downr = pp.tile([P, 3 * KT], f32)
            acc_sb = pp.tile([P, n_slots + (BANDT - 1) * KT], f32)
            ones = pp.tile([P, 1], bf16)
            identity = pp.tile([P, P], bf16)
            trash = pp.tile([P, CH], bf16)
            acc = accp.tile([P, n_slots + (BANDT - 1) * KT], f32)

            nc.vector.memset(ones[:], 1.0)
            masks.make_identity(nc, identity[:])
            nc.scalar.dma_start(out=mt[:], in_=m_h[:, :])

            with (
                tc.tile_pool(name="xt", bufs=3) as xtp,
                tc.tile_pool(name="tp", bufs=1, space="PSUM") as tpp,
                tc.tile_pool(name="mm", bufs=2, space="PSUM") as mmp,
                tc.tile_pool(name="ep", bufs=4) as ep,
                tc.tile_pool(name="emp", bufs=2) as emp,
            ):
                pending = []  # delayed colsum emission for PE overlap

                def emit_pending():
                    for fn in pending:
                        fn()
                    pending.clear()

                def emit_mega(m):
                    xt = xtp.tile([P, MEGA, D], bf16, tag="xt", name=f"xt{m}")
                    dma_eng = (nc.sync, nc.gpsimd, nc.scalar)[m % 3]
                    dma_eng.dma_start(
                        out=xt[:],
                        in_=x_h[m * MEGA * P : (m + 1) * MEGA * P, :].rearrange(
                            "(g p) d -> p g d", p=P
                        ),
                    )
                    pt = tpp.tile([P, MEGA * P], bf16, tag="pt", name=f"pt{m}")
                    for g in range(MEGA):
                        nc.tensor.transpose(
                            out=pt[:, g * P : (g + 1) * P],
                            in_=xt[:, g, :],
                            identity=identity[:],
                        )
                    nc.vector.tensor_copy(
                        out=xnT[:, m * MEGA * P : (m + 1) * MEGA * P], in_=pt[:]
                    )

                def emit_chunk(k, j, cw, mms):
                    lhsT = xnT[:, 1024 * k : 1024 * k + P]
                    ps = mmp.tile([P, CH], f32, tag="ps", name=f"ps{k}_{j}")
                    for off, col, w in mms:
                        nc.tensor.matmul(
                            out=ps[:, off : off + w],
                            lhsT=lhsT,
                            rhs=xnT[:, col : col + w],
                            start=True,
                            stop=True,
                        )
                    emit_pending()
                    e = ep.tile([P, CH], bf16, tag="e", name=f"e{k}_{j}")
                    dcol = downr[:, 3 * k + j : 3 * k + j + 1]
                    nc.scalar.activation(
                        out=e[:, :cw],
                        in_=ps[:, :cw],
                        func=AF.Exp,
                        scale=1.0 / TAU,
                        accum_out=dcol if j != 1 else None,
                    )
                    if j == 1:
                        nc.vector.tensor_reduce(
                            out=dcol, in_=e[:, :cw], axis=AX.X, op=OP.add
                        )
                    if j == 0:
                        em = emp.tile([P, BW], bf16, tag="em", name=f"em{k}")
                        nc.vector.scalar_tensor_tensor(
                            out=em[:],
                            in0=mt[:, k * BW : (k + 1) * BW],
                            scalar=1.0,
                            in1=e[:, :BW],
                            op0=OP.mult,
                            op1=OP.mult,
                            accum_out=topr[:, k : k + 1],
                        )

                        def top_cols(k=k, em=em):
                            for w in range(1, BANDT):
                                s = n_slots + (BANDT - 1) * k + w - 1
                                nc.tensor.matmul(
                                    out=acc[:, s : s + 1],
                                    lhsT=em[:, w * P : (w + 1) * P],
                                    rhs=ones[:],
                                    start=True,
                                    stop=True,
                                )

                        pending.append(top_cols)

                    def down_cols(k=k, j=j, cw=cw, e=e):
                        for wo in range(0, cw, P):
                            w = (j * CH + wo) // P
                            if w == 0:
                                continue  # diagonal tile: rows cover it
                            s = slot_of[(k, w)]
                            nc.tensor.matmul(
                                out=acc[:, s : s + 1],
                                lhsT=e[:, wo : wo + P],
                                rhs=ones[:],
                                start=True,
                                stop=True,
                            )

                    pending.append(down_cols)

                for m in range(NMEGA):
                    emit_mega(m)
                    for k, j, cw, mms in sorted(sched[m]):
                        emit_chunk(k, j, cw, mms)
                emit_pending()

            nc.vector.tensor_copy(out=acc_sb[:], in_=acc[:])
            nc.sync.dma_start(out=topr_h[:, :], in_=topr[:])
            nc.sync.dma_start(out=downr_h[:, :], in_=downr[:])
            nc.sync.dma_start(out=downc_h[:, :], in_=acc_sb[:, :n_slots])
            nc.sync.dma_start(
                out=topc_h[:, :], in_=acc_sb[:, n_slots : n_slots + (BANDT - 1) * KT]
            )
    nc.compile()
    return nc


def _get_program():
    global _PROGRAM
    if _PROGRAM is None:
        _PROGRAM = _build_program()
    return _PROGRAM


def make_in_maps(x, y):
    x = np.asarray(x, dtype=np.float32)
    y = np.asarray(y)
    perm = np.argsort(y, kind="stable")
    xs = np.ascontiguousarray(x[perm])
    xs = xs / np.linalg.norm(xs, axis=-1, keepdims=True)
    xs = xs.astype(ml_dtypes.bfloat16)
    ys = np.asarray(y)[perm].astype(np.int64)

    # class spans must fit the BANDT-tile mask band
    uniq = np.unique(ys)
    starts = np.searchsorted(ys, uniq, side="left")
    ends = np.searchsorted(ys, uniq, side="right")
    assert np.max((ends - 1) // P - starts // P) <= BANDT - 1, (
        "class span exceeds mask band; raise BANDT"
    )

    in_maps = []
    for c in range(NCORES):
        rot = P * c
        xr = np.ascontiguousarray(np.roll(xs, -rot, axis=0))
        yl = np.roll(ys, -rot)
        m = np.zeros((P, KT * BW), dtype=ml_dtypes.bfloat16)
        for k in range(KT):
            rcls = yl[1024 * k : 1024 * k + P]          # own tile k rows
            ccls = yl[(1024 * k + np.arange(BW)) % N]   # band cols
            m[:, k * BW : (k + 1) * BW] = (
                rcls[:, None] == ccls[None, :]
            ).astype(ml_dtypes.bfloat16)
        in_maps.append({"x": xr, "m": m})
    return in_maps


def finalize(results):
    """results: list of 8 dicts with topr/downr/downc/topc -> scalar loss."""
    slot_kw = []
    for k in range(KT):
        for w in range(1, _win_tiles(8 * k)):
            slot_kw.append((k, w))

    down = np.zeros(N, np.float64)
    top = np.zeros(N, np.float64)
    for c, r in enumerate(results):
        topr = np.asarray(r["topr"], np.float64)
        downr = np.asarray(r["downr"], np.float64)
        downc = np.asarray(r["downc"], np.float64)
        topc = np.asarray(r["topc"], np.float64)
        p = np.arange(P)
        for k in range(KT):
            gl = P * (8 * k + c) + p
            down[gl] += downr[:, 3 * k : 3 * k + 3].sum(axis=1)
            top[gl] += topr[:, k]
            for w in range(1, BANDT):
                vloc = 8 * k + w
                gl2 = P * ((vloc + c) % NT) + p
                top[gl2] += topc[:, (BANDT - 1) * k + w - 1]
        for s, (k, w) in enumerate(slot_kw):
            vloc = (8 * k + w) % NT
            gl = P * ((vloc + c) % NT) + p
            down[gl] += downc[:, s]
    return np.float32(np.mean(np.log(down) - np.log(top)))


def kernel(x, y):
    from concourse.bass_utils import run_bass_kernel_spmd

    nc = _get_program()
    in_maps = make_in_maps(x, y)
    res = run_bass_kernel_spmd(nc, in_maps, list(range(NCORES)))
    return finalize(res.results)
